# revision 15
# baseline (speedup 1.0000x reference)
"""Trainium2 Bass kernel for a single causal attention head.

Problem: x [8, 2048, 1024] f32, Wq/Wk/Wv [1024, 64] f32 ->
         out [8, 2048, 64] f32  (causal softmax(q k^T / 8) v per batch)

Strategy: data-parallel over batch -- core b computes batch element b,
no collectives. Per core, a column-pipelined flash-style kernel over
4 t-slices of 512.

Trace-driven structure (v2):
  * x/W cast to bf16 on the host; every DMA goes through HWDGE.
  * x j0 lands in [1,1,2,4] e-chunks so the first projection matmul
    starts ~1.3us after its trigger; j1-j3 are single 1MB DMAs (fewer
    Sync trigger slots + completion receipts).  Output stores moved off
    Sync onto the otherwise-idle GpSimd DGE so they never delay x
    triggers.
  * 6 warm matmuls (was 12) bridge the first DMA wait; finer x chunks
    close the warmup->projection gap that used to drop the HAM clock
    gate to 4/8 mid-kernel.
  * the qk -> [kT;qT] partition swap is split into two concurrent
    64-row PE matmuls against a stacked identity, each gated on only
    half of the PSUM->SBUF cast, shortening the j0 critical path.
  * causal masking moved off the PE: exp runs on raw scores and the
    16 diagonal 128x128 windows are zeroed after exp by DVE
    tensor_mul with a 0/1 bf16 mask (4x DVE mode, ~0.1us each).  This
    removes 16 mask matmuls + 16 LDWEIGHTS from the PE stream and
    lets exp fire right after each score pair.
  * scores as row-tiled concurrent pairs (explicit tile_position), exp
    batched [128, 2, 512] across two PSUM banks, steep staircase pair
    only computes [256:512].
  * output stored bf16 (halves store bytes; adds ~0.4% rounding, well
    inside the 2e-2 gate), windows stream out via GpSimd as soon as
    normalized; the final window goes through ScalarE's HWDGE so the
    last two completion receipts overlap.

Layouts (bf16 compute, f32 PSUM accumulation):
  xp     [4, 128, 8, 512] bf16 DRAM (host-marshalled x^T slices)
  wp     [128, 1728] bf16: per e-chunk [Wq|Wk|Wv] blocks, then the
         stacked identity (swap) and 0/1 causal mask constants
  qT|kT  [128, T] (q rows 0-63, k rows 64-127), one M=128 matmul chain
  qk2    [kT; qT] partition-swap of qk (two concurrent 64-row matmuls)
  scores [s-chunk 128, t 512] pairs into [128, 2, 512] PSUM groups
  exp    bf16 [128, 2, 512] tiles; diag windows masked by DVE mult
  PV     out[t 128, 65] = expT_block^T @ [v | 1]; column 64 gives the
         softmax denominator; normalize via reciprocal + tensor_scalar.
  out    [4, 4, 128, 64] bf16 DRAM; (j, c, p) lexicographic == t.
"""

import sys
from contextlib import ExitStack

sys.path.insert(0, "/opt/trn_rl_repo")

import numpy as np
import ml_dtypes

import concourse.bass as bass
import concourse.tile as tile
from concourse import bacc, mybir
from concourse.bass_utils import run_bass_kernel_spmd

B, T, E, H = 8, 2048, 1024, 64
NCORES = 8
TJ = 512            # t-slice width (score tile free dim)
NJ = T // TJ        # 4 columns
NWARM = 7           # dummy matmuls bridging the first x-chunk DMA wait
BF16 = mybir.dt.bfloat16
F32 = mybir.dt.float32
WCW = 1728          # wp width: 1536 weights + 64 idn2 + 128 mask01


def build_kernel(tc: "tile.TileContext", out: bass.AP, xp_dram: bass.AP,
                 wp_dram: bass.AP):
    nc = tc.nc
    EXP = mybir.ActivationFunctionType.Exp

    ctx = ExitStack()
    const = ctx.enter_context(tc.tile_pool(name="const", bufs=1))
    expp = ctx.enter_context(tc.tile_pool(name="expp", bufs=20))
    outp = ctx.enter_context(tc.tile_pool(name="outp", bufs=8))
    small = ctx.enter_context(tc.tile_pool(name="small", bufs=8))
    ps_qk = ctx.enter_context(tc.tile_pool(name="ps_qk", bufs=1, space="PSUM"))
    ps_v = ctx.enter_context(tc.tile_pool(name="ps_v", bufs=1, space="PSUM"))
    ps_s = ctx.enter_context(tc.tile_pool(name="ps_s", bufs=2, space="PSUM"))
    ps_o = ctx.enter_context(tc.tile_pool(name="ps_o", bufs=2, space="PSUM"))

    # PE warm-up: zero tile via GpSimd memset, then NWARM dummy matmuls
    # that run during the w/x DMA wait so the PE clock ramps before real
    # work arrives.  More filler warms are interleaved into the first
    # projection chain (emit_warm) so the HAM activity monitor never sees
    # a gap while the chain is paced by x-chunk DMA completions.
    warm = const.tile([128, TJ], BF16, tag="warm")
    nc.gpsimd.memset(warm[:], 0.0)
    warm_ps = ps_s.tile([128, 2, TJ], F32, tag="s", name="warm_ps")

    def emit_warm():
        nc.tensor.matmul(warm_ps[:, 0, :], warm[:, 0:128], warm[:],
                         start=True, stop=True)

    for k in range(NWARM):
        emit_warm()

    # Weights + constants in one DMA on the ScalarE DGE queue so the x
    # slice-0 chunks head the Sync queue (wc is 432 KB and would delay
    # the first projection chunk by ~1.5us otherwise).
    wc_sb = const.tile([128, WCW], BF16, tag="wc")
    nc.scalar.dma_start(wc_sb[:], wp_dram[:])
    # flat views: W chunk ec lives at cols [ec*192, ec*192+192) as [Wq|Wk|Wv]
    wqk = [wc_sb[:, ec * 192:ec * 192 + 128] for ec in range(8)]
    wv = [wc_sb[:, ec * 192 + 128:ec * 192 + 192] for ec in range(8)]
    idn2 = wc_sb[:, 1536:1600]     # [I64; I64] stacked, for the qk swap
    mask01 = wc_sb[:, 1600:1728]   # [128,128] 0/1 bf16, keep cols >= row

    # x slices: all four stay resident (32 KB/partition total).  Loads are
    # split across the Sync and Vector DGE queues so two queues pull from
    # HBM concurrently; j0 lands in fine e-chunks, interleaved over both
    # queues, so the projection chain starts on chunk 0 and is paced by
    # chunk completion.
    x_tiles = []
    for j in range(NJ):
        x_tiles.append(const.tile([128, 8, TJ], BF16, tag=f"x{j}", name=f"x{j}"))

    # (engine, j, a, b) in per-queue emission order: each queue's list is
    # FIFO, the two queues run concurrently.  Only Sync and Scalar have
    # HWDGE; the Scalar queue legs all fire before its exp stream starts.
    plan = [
        (nc.sync,   0, 0, 1), (nc.scalar, 0, 1, 2),
        (nc.sync,   0, 2, 4), (nc.scalar, 0, 4, 6),
        (nc.sync,   0, 6, 8),
        (nc.sync,   1, 0, 4), (nc.scalar, 1, 4, 8),
        (nc.sync,   2, 0, 4), (nc.scalar, 2, 4, 8),
        (nc.sync,   3, 0, 4), (nc.scalar, 3, 4, 8),
    ]
    for eng, j, a, b in plan:
        eng.dma_start(x_tiles[j][:, a:b, :], xp_dram[j][:, a:b, :])

    qk_tiles = []   # per column: [128, TJ] bf16, rows 0:64 = qT, 64:128 = kT
    qk2_tiles = []  # per column: [128, TJ] bf16, rows 0:64 = kT, 64:128 = qT
    v_tiles = []    # per column: [128, 4, H+1] bf16 ([v | ones])
    for j in range(NJ):
        qk_tiles.append(const.tile([128, TJ], BF16, tag=f"qk{j}", name=f"qk{j}"))
        qk2_tiles.append(const.tile([128, TJ], BF16, tag=f"qk2{j}", name=f"qk2{j}"))
        v_tiles.append(const.tile([128, 4, H + 1], BF16, tag=f"v{j}", name=f"v{j}"))

    etiles = {}     # (j, b) -> exp pair tile [128, 2, TJ]

    def emit_qk(j):
        """q/k projection + partition swap for column j.

        On the critical first column, filler warm matmuls are threaded
        between the DMA-paced chunk matmuls to keep the HAM clock
        monitor fed."""
        qk_j, qk2_j = qk_tiles[j], qk2_tiles[j]
        xsl = x_tiles[j]

        # combined q/k projection: psum[0:64]=qT, [64:128]=kT
        psq = ps_qk.tile([128, TJ], F32, tag="qk", name=f"psq{j}")
        for ec in range(8):
            nc.tensor.matmul(
                psq[:], wqk[ec], xsl[:, ec, :],
                start=(ec == 0), stop=(ec == 7),
            )
            if j == 0 and ec <= 3:
                emit_warm()
        nc.vector.tensor_copy(qk_j[:], psq[:])

        # partition swap on the PE as two concurrent 64-row matmuls:
        # psq2[0:64] = kT (reads qk[64:128]), psq2[64:128] = qT.
        psq2 = ps_qk.tile([128, TJ], F32, tag="qk", name=f"psq2_{j}")
        nc.tensor.matmul(psq2[0:64, :], idn2[64:128, :], qk_j[64:128, :],
                         start=True, stop=True, tile_position=(64, 0))
        nc.tensor.matmul(psq2[64:128, :], idn2[0:64, :], qk_j[0:64, :],
                         start=True, stop=True, tile_position=(0, 64))
        nc.vector.tensor_copy(qk2_j[:], psq2[:])

    def emit_v(j):
        """v projection for s-chunks 4j..4j+3 into one PSUM bank.

        Emitted AFTER column j's score pairs: v is only needed by the
        (much later) PV windows, and on the PE's in-order stream it must
        not delay the score pairs that feed ScalarE."""
        v_j = v_tiles[j]
        xsl = x_tiles[j]
        nc.vector.memset(v_j[:, :, H], 1.0)
        psv = ps_v.tile([128, 4, H], F32, tag="v", name=f"psv{j}")
        for c in range(4):
            for ec in range(8):
                nc.tensor.matmul(
                    psv[:, c, :], xsl[:, ec, c * 128:(c + 1) * 128],
                    wv[ec], start=(ec == 0), stop=(ec == 7),
                )
        nc.vector.tensor_copy(v_j[:, :, 0:H], psv[:])

    def emit_score_pair(j, b):
        """score tiles i=2b, 2b+1 of column j as a row-tiled pair + one exp.

        No mask matmuls: exp runs on raw scores and the diagonal windows
        are zeroed afterwards by DVE tensor_mul with the 0/1 mask."""
        grp = ps_s.tile([128, 2, TJ], F32, tag="s", name=f"pss{j}_{b}")
        f0s = []
        for slot in range(2):
            i = 2 * b + slot
            r = i - 4 * j          # r >= 0 -> staircase block
            f0 = 128 * r if r > 0 else 0
            f0s.append(f0)
            isl = slice((i % 4) * 128, (i % 4 + 1) * 128)
            if slot == 0:
                # PE rows 0-63: kT from the swapped copy, qT native
                nc.tensor.matmul(
                    grp[:, 0, f0:], qk2_tiles[i // 4][0:64, isl],
                    qk_tiles[j][0:64, f0:], start=True, stop=True,
                    tile_position=(0, 0),
                )
            else:
                # PE rows 64-127: kT native, qT from the swapped copy
                nc.tensor.matmul(
                    grp[:, 1, f0:], qk_tiles[i // 4][64:128, isl],
                    qk2_tiles[j][64:128, f0:], start=True, stop=True,
                    tile_position=(64, 0),
                )
        e = expp.tile([128, 2, TJ], BF16, tag="e", name=f"e{j}_{b}")
        # batched exp across both PSUM banks; the steep staircase pair
        # (f0 >= 256 in both slots) only needs the [256:] columns.
        t0 = 256 if min(f0s) >= 256 else 0
        nc.scalar.activation(e[:, :, t0:], grp[:, :, t0:], EXP, scale=0.125)
        # zero the strict-upper triangle of each diagonal 128-wide window
        for slot in range(2):
            r = 2 * b + slot - 4 * j
            if r >= 0:
                f0 = 128 * r if r > 0 else 0
                nc.vector.tensor_mul(
                    e[:, slot, f0:f0 + 128], e[:, slot, f0:f0 + 128], mask01
                )
        etiles[(j, b)] = e

    def emit_pv_mms(j, c):
        m = 4 * j + c
        po = ps_o.tile([128, H + 1], F32, tag="po", name=f"po{j}_{c}")
        for i in range(m + 1):
            nc.tensor.matmul(
                po[:], etiles[(j, i // 2)][:, i % 2, c * 128:(c + 1) * 128],
                v_tiles[i // 4][:, i % 4, :],
                start=(i == 0), stop=(i == m),
            )
        return po

    def emit_pv_finish(j, c, po, engine=None):
        rec = small.tile([128, 1], F32, tag="rec", name=f"rec{j}_{c}")
        nc.vector.reciprocal(rec[:], po[:, H:H + 1])
        osb = outp.tile([128, H], BF16, tag="o", name=f"osb{j}_{c}")
        nc.vector.tensor_scalar_mul(osb[:], po[:, 0:H], rec[:])
        (engine or nc.sync).dma_start(out[j][c], osb[:])

    def emit_pv_window(j, c):
        """PV + normalize + store for output window c of column j."""
        emit_pv_finish(j, c, emit_pv_mms(j, c))

    # Emission order == static-scheduler priority: qk projections + score
    # pairs first (they feed ScalarE, the bottleneck of the steady
    # phase); each column's v projection is demoted to just after its
    # score pairs; PV windows after in chronological order -- the
    # scheduler slots PV matmuls into the exp-turnaround stalls on its
    # own.
    # v_j is demoted below column j+1's score pairs: on the in-order PE
    # stream, v matmuls emitted earlier would delay the score pairs that
    # feed ScalarE (the steady-phase bottleneck).
    for j in range(NJ):
        emit_qk(j)
        for b in range(2 * j + 2):
            emit_score_pair(j, b)
        if j >= 1:
            emit_v(j - 1)
    emit_v(NJ - 1)
    for j in range(NJ - 1):
        for c in range(4):
            emit_pv_window(j, c)
    emit_pv_window(NJ - 1, 0)
    emit_pv_window(NJ - 1, 1)
    po2 = emit_pv_mms(NJ - 1, 2)
    po3 = emit_pv_mms(NJ - 1, 3)
    emit_pv_finish(NJ - 1, 2, po2)
    # last store via ScalarE HWDGE (idle by now) so the two final
    # completion receipts overlap instead of queueing
    emit_pv_finish(NJ - 1, 3, po3, engine=nc.scalar)

    ctx.close()


_NC_CACHE = None


def build_nc():
    global _NC_CACHE
    if _NC_CACHE is not None:
        return _NC_CACHE
    nc = bacc.Bacc(
        "TRN2", target_bir_lowering=False, debug=False,
        enable_asserts=False, num_devices=NCORES,
    )
    xp_dram = nc.dram_tensor("xp", [NJ, 128, 8, TJ], BF16, kind="ExternalInput").ap()
    wp_dram = nc.dram_tensor("wp", [128, WCW], BF16, kind="ExternalInput").ap()
    out = nc.dram_tensor("out", [NJ, 4, 128, H], BF16, kind="ExternalOutput").ap()
    with tile.TileContext(nc) as tc:
        build_kernel(tc, out, xp_dram, wp_dram)
    nc.finalize()
    _NC_CACHE = nc
    return nc


def _const_cst():
    p = np.arange(128)
    idn2 = np.tile(np.eye(64, dtype=np.float32), (2, 1))        # [128, 64]
    mask01 = (p[:, None] <= p[None, :]).astype(np.float32)      # [128, 128]
    return np.ascontiguousarray(
        np.concatenate([idn2, mask01], axis=1)
    ).astype(ml_dtypes.bfloat16)


def _marshal(x_b: np.ndarray):
    # xp[j, p, ec, t'] = x[j*TJ + t', ec*128 + p], cast bf16
    return np.ascontiguousarray(
        x_b.reshape(NJ, TJ, 8, 128).transpose(0, 3, 2, 1)
    ).astype(ml_dtypes.bfloat16)


def _install_profile_hook():
    """The agent image lacks ``antenv.axon_hooks``; inject a shim so
    run_bass_kernel_spmd(trace=True) can reach the axon NTFF profiler."""
    import types

    if "antenv.axon_hooks" not in sys.modules:
        mod = types.ModuleType("antenv.axon_hooks")
        holder = {}
        mod.set_axon_ntff_profile_hook = lambda h: holder.__setitem__("h", h)
        mod.get_axon_ntff_profile_hook = lambda: holder.get("h")
        sys.modules["antenv.axon_hooks"] = mod
    from trn_agent_boot.trn_boot import _ntff_profile_via_ctypes

    hook = _ntff_profile_via_ctypes("/opt/axon/libaxon_pjrt.so")
    sys.modules["antenv.axon_hooks"].set_axon_ntff_profile_hook(hook)
    # no fish bucket in this container -- keep artifacts local
    from concourse import bass_utils as bu

    bu.upload_artifacts = lambda tmpdir: tmpdir


def run(inputs: dict, trace: bool = False, tmpdir: str | None = None):
    """Returns (out [8, 2048, 64] f32, exec_time_ns or None)."""
    x = np.asarray(inputs["x"], dtype=np.float32)
    # wp[p, ec, r, h] = W_r[ec*128 + p, h], cast bf16
    wqkv = np.stack([np.asarray(inputs["Wq"]), np.asarray(inputs["Wk"]),
                     np.asarray(inputs["Wv"])]).astype(np.float32)
    w_pre = np.ascontiguousarray(
        wqkv.reshape(3, 8, 128, H).transpose(2, 1, 0, 3)
    ).astype(ml_dtypes.bfloat16)
    nc = build_nc()
    if trace:
        _install_profile_hook()
    wc = np.concatenate(
        [w_pre.reshape(128, 1536), _const_cst()], axis=1
    )
    in_maps = [{"xp": _marshal(x[b]), "wp": wc} for b in range(B)]
    res = run_bass_kernel_spmd(
        nc, in_maps, core_ids=list(range(NCORES)), trace=trace, tmpdir=tmpdir
    )
    # out[j, c, p, h]: (j, c, p) lexicographic == t = j*512 + c*128 + p
    out = np.stack([
        res.results[b]["out"].reshape(T, H) for b in range(B)
    ]).astype(np.float32)
    return out, res.exec_time_ns


def kernel(**inputs) -> np.ndarray:
    out, _ = run(inputs)
    return out


if __name__ == "__main__":
    rng = np.random.default_rng(0)
    ins = {
        "x": rng.standard_normal((B, T, E), dtype=np.float32),
        "Wq": rng.uniform(-1 / 32, 1 / 32, (E, H)).astype(np.float32),
        "Wk": rng.uniform(-1 / 32, 1 / 32, (E, H)).astype(np.float32),
        "Wv": rng.uniform(-1 / 32, 1 / 32, (E, H)).astype(np.float32),
    }
    o, ns = run(ins, trace=False)
    print("out", o.shape, o.dtype, "exec_ns", ns)


# revision 18
# speedup vs baseline: 1.0059x; 1.0059x over previous
"""Trainium2 Bass kernel for a single causal attention head.

Problem: x [8, 2048, 1024] f32, Wq/Wk/Wv [1024, 64] f32 ->
         out [8, 2048, 64] f32  (causal softmax(q k^T / 8) v per batch)

Strategy: data-parallel over batch -- core b computes batch element b,
no collectives. Per core, a column-pipelined flash-style kernel over
4 t-slices of 512.

Trace-driven structure (v2):
  * x/W cast to bf16 on the host; every DMA goes through HWDGE.
  * x j0 lands in [1,1,2,4] e-chunks so the first projection matmul
    starts ~1.3us after its trigger; j1-j3 are single 1MB DMAs (fewer
    Sync trigger slots + completion receipts).  Output stores moved off
    Sync onto the otherwise-idle GpSimd DGE so they never delay x
    triggers.
  * 6 warm matmuls (was 12) bridge the first DMA wait; finer x chunks
    close the warmup->projection gap that used to drop the HAM clock
    gate to 4/8 mid-kernel.
  * the qk -> [kT;qT] partition swap is split into two concurrent
    64-row PE matmuls against a stacked identity, each gated on only
    half of the PSUM->SBUF cast, shortening the j0 critical path.
  * causal masking moved off the PE: exp runs on raw scores and the
    16 diagonal 128x128 windows are zeroed after exp by DVE
    tensor_mul with a 0/1 bf16 mask (4x DVE mode, ~0.1us each).  This
    removes 16 mask matmuls + 16 LDWEIGHTS from the PE stream and
    lets exp fire right after each score pair.
  * scores as row-tiled concurrent pairs (explicit tile_position), exp
    batched [128, 2, 512] across two PSUM banks, steep staircase pair
    only computes [256:512].
  * output stored bf16 (halves store bytes; adds ~0.4% rounding, well
    inside the 2e-2 gate), windows stream out via GpSimd as soon as
    normalized; the final window goes through ScalarE's HWDGE so the
    last two completion receipts overlap.

Layouts (bf16 compute, f32 PSUM accumulation):
  xp     [4, 128, 8, 512] bf16 DRAM (host-marshalled x^T slices)
  wp     [128, 1728] bf16: per e-chunk [Wq|Wk|Wv] blocks, then the
         stacked identity (swap) and 0/1 causal mask constants
  qT|kT  [128, T] (q rows 0-63, k rows 64-127), one M=128 matmul chain
  qk2    [kT; qT] partition-swap of qk (two concurrent 64-row matmuls)
  scores [s-chunk 128, t 512] pairs into [128, 2, 512] PSUM groups
  exp    bf16 [128, 2, 512] tiles; diag windows masked by DVE mult
  PV     out[t 128, 65] = expT_block^T @ [v | 1]; column 64 gives the
         softmax denominator; normalize via reciprocal + tensor_scalar.
  out    [4, 4, 128, 64] bf16 DRAM; (j, c, p) lexicographic == t.
"""

import sys
from contextlib import ExitStack

sys.path.insert(0, "/opt/trn_rl_repo")

import numpy as np
import ml_dtypes

import concourse.bass as bass
import concourse.tile as tile
from concourse import bacc, mybir
from concourse.bass_utils import run_bass_kernel_spmd

B, T, E, H = 8, 2048, 1024, 64
NCORES = 8
TJ = 512            # t-slice width (score tile free dim)
NJ = T // TJ        # 4 columns
NWARM = 7           # dummy matmuls bridging the first x-chunk DMA wait
BF16 = mybir.dt.bfloat16
F32 = mybir.dt.float32
WCW = 1728          # wp width: 1536 weights + 64 idn2 + 128 mask01


def build_kernel(tc: "tile.TileContext", out: bass.AP, xp_dram: bass.AP,
                 wp_dram: bass.AP):
    nc = tc.nc
    EXP = mybir.ActivationFunctionType.Exp

    ctx = ExitStack()
    const = ctx.enter_context(tc.tile_pool(name="const", bufs=1))
    expp = ctx.enter_context(tc.tile_pool(name="expp", bufs=20))
    outp = ctx.enter_context(tc.tile_pool(name="outp", bufs=8))
    small = ctx.enter_context(tc.tile_pool(name="small", bufs=8))
    ps_qk = ctx.enter_context(tc.tile_pool(name="ps_qk", bufs=1, space="PSUM"))
    ps_v = ctx.enter_context(tc.tile_pool(name="ps_v", bufs=1, space="PSUM"))
    ps_s = ctx.enter_context(tc.tile_pool(name="ps_s", bufs=2, space="PSUM"))
    ps_o = ctx.enter_context(tc.tile_pool(name="ps_o", bufs=2, space="PSUM"))

    # PE warm-up: zero tile via GpSimd memset, then NWARM dummy matmuls
    # that run during the w/x DMA wait so the PE clock ramps before real
    # work arrives.  More filler warms are interleaved into the first
    # projection chain (emit_warm) so the HAM activity monitor never sees
    # a gap while the chain is paced by x-chunk DMA completions.
    warm = const.tile([128, TJ], BF16, tag="warm")
    nc.gpsimd.memset(warm[:], 0.0)
    warm_ps = ps_s.tile([128, 2, TJ], F32, tag="s", name="warm_ps")

    def emit_warm():
        nc.tensor.matmul(warm_ps[:, 0, :], warm[:, 0:128], warm[:],
                         start=True, stop=True)

    for k in range(NWARM):
        emit_warm()

    # Weights + constants ride the ScalarE DGE queue (x slice-0 chunks
    # head the Sync queue), split in three so the x legs interleaved on
    # this queue are not stuck behind the whole 432 KB block.
    wc_sb = const.tile([128, WCW], BF16, tag="wc")
    # flat views: W chunk ec lives at cols [ec*192, ec*192+192) as [Wq|Wk|Wv]
    wqk = [wc_sb[:, ec * 192:ec * 192 + 128] for ec in range(8)]
    wv = [wc_sb[:, ec * 192 + 128:ec * 192 + 192] for ec in range(8)]
    idn2 = wc_sb[:, 1536:1600]     # [I64; I64] stacked, for the qk swap
    mask01 = wc_sb[:, 1600:1728]   # [128,128] 0/1 bf16, keep cols >= row

    # x slices: all four stay resident (32 KB/partition total).  Loads are
    # split across the Sync and Vector DGE queues so two queues pull from
    # HBM concurrently; j0 lands in fine e-chunks, interleaved over both
    # queues, so the projection chain starts on chunk 0 and is paced by
    # chunk completion.
    x_tiles = []
    for j in range(NJ):
        x_tiles.append(const.tile([128, 8, TJ], BF16, tag=f"x{j}", name=f"x{j}"))

    # (engine, j, a, b) in per-queue emission order: each queue's list is
    # FIFO, the two queues run concurrently and together saturate the
    # DMA fabric.  Only Sync and Scalar have HWDGE; the Scalar queue
    # legs all fire before its exp stream starts.  The wc pieces are
    # interleaved so scalar-queue x legs land early: wqk[0:4] | ec1 |
    # wqk[4:8] | ec[4:6] | idn2+mask.
    nc.scalar.dma_start(wc_sb[:, 0:768], wp_dram[:, 0:768])
    plan = [
        (nc.sync,   0, 0, 1), (nc.scalar, 0, 1, 2),
        (nc.sync,   0, 2, 4),
    ]
    for eng, j, a, b in plan:
        eng.dma_start(x_tiles[j][:, a:b, :], xp_dram[j][:, a:b, :])
    nc.scalar.dma_start(wc_sb[:, 768:1536], wp_dram[:, 768:1536])
    plan = [
        (nc.scalar, 0, 4, 6), (nc.sync,   0, 6, 8),
    ]
    for eng, j, a, b in plan:
        eng.dma_start(x_tiles[j][:, a:b, :], xp_dram[j][:, a:b, :])
    nc.scalar.dma_start(wc_sb[:, 1536:WCW], wp_dram[:, 1536:WCW])
    plan = [
        (nc.sync,   1, 0, 4), (nc.scalar, 1, 4, 8),
        (nc.sync,   2, 0, 4), (nc.scalar, 2, 4, 8),
        (nc.sync,   3, 0, 4), (nc.scalar, 3, 4, 8),
    ]
    for eng, j, a, b in plan:
        eng.dma_start(x_tiles[j][:, a:b, :], xp_dram[j][:, a:b, :])

    qk_tiles = []   # per column: [128, TJ] bf16, rows 0:64 = qT, 64:128 = kT
    qk2_tiles = []  # per column: [128, TJ] bf16, rows 0:64 = kT, 64:128 = qT
    v_tiles = []    # per column: [128, 4, H+1] bf16 ([v | ones])
    for j in range(NJ):
        qk_tiles.append(const.tile([128, TJ], BF16, tag=f"qk{j}", name=f"qk{j}"))
        qk2_tiles.append(const.tile([128, TJ], BF16, tag=f"qk2{j}", name=f"qk2{j}"))
        v_tiles.append(const.tile([128, 4, H + 1], BF16, tag=f"v{j}", name=f"v{j}"))

    etiles = {}     # (j, b) -> exp pair tile [128, 2, TJ]

    def emit_qk(j):
        """q/k projection + partition swap for column j.

        On the critical first column, filler warm matmuls are threaded
        between the DMA-paced chunk matmuls to keep the HAM clock
        monitor fed."""
        qk_j, qk2_j = qk_tiles[j], qk2_tiles[j]
        xsl = x_tiles[j]

        # combined q/k projection: psum[0:64]=qT, [64:128]=kT
        psq = ps_qk.tile([128, TJ], F32, tag="qk", name=f"psq{j}")
        for ec in range(8):
            nc.tensor.matmul(
                psq[:], wqk[ec], xsl[:, ec, :],
                start=(ec == 0), stop=(ec == 7),
            )
            if j == 0 and ec <= 3:
                emit_warm()
        nc.vector.tensor_copy(qk_j[:], psq[:])

        # partition swap on the PE as two concurrent 64-row matmuls:
        # psq2[0:64] = kT (reads qk[64:128]), psq2[64:128] = qT.
        psq2 = ps_qk.tile([128, TJ], F32, tag="qk", name=f"psq2_{j}")
        nc.tensor.matmul(psq2[0:64, :], idn2[64:128, :], qk_j[64:128, :],
                         start=True, stop=True, tile_position=(64, 0))
        nc.tensor.matmul(psq2[64:128, :], idn2[0:64, :], qk_j[0:64, :],
                         start=True, stop=True, tile_position=(0, 64))
        nc.vector.tensor_copy(qk2_j[:], psq2[:])

    def emit_v(j):
        """v projection for s-chunks 4j..4j+3 into one PSUM bank.

        Emitted AFTER column j's score pairs: v is only needed by the
        (much later) PV windows, and on the PE's in-order stream it must
        not delay the score pairs that feed ScalarE."""
        v_j = v_tiles[j]
        xsl = x_tiles[j]
        nc.vector.memset(v_j[:, :, H], 1.0)
        psv = ps_v.tile([128, 4, H], F32, tag="v", name=f"psv{j}")
        for c in range(4):
            for ec in range(8):
                nc.tensor.matmul(
                    psv[:, c, :], xsl[:, ec, c * 128:(c + 1) * 128],
                    wv[ec], start=(ec == 0), stop=(ec == 7),
                )
        nc.vector.tensor_copy(v_j[:, :, 0:H], psv[:])

    def emit_score_pair(j, b):
        """score tiles i=2b, 2b+1 of column j as a row-tiled pair + one exp.

        No mask matmuls: exp runs on raw scores and the diagonal windows
        are zeroed afterwards by DVE tensor_mul with the 0/1 mask."""
        grp = ps_s.tile([128, 2, TJ], F32, tag="s", name=f"pss{j}_{b}")
        f0s = []
        for slot in range(2):
            i = 2 * b + slot
            r = i - 4 * j          # r >= 0 -> staircase block
            f0 = 128 * r if r > 0 else 0
            f0s.append(f0)
            isl = slice((i % 4) * 128, (i % 4 + 1) * 128)
            if slot == 0:
                # PE rows 0-63: kT from the swapped copy, qT native
                nc.tensor.matmul(
                    grp[:, 0, f0:], qk2_tiles[i // 4][0:64, isl],
                    qk_tiles[j][0:64, f0:], start=True, stop=True,
                    tile_position=(0, 0),
                )
            else:
                # PE rows 64-127: kT native, qT from the swapped copy
                nc.tensor.matmul(
                    grp[:, 1, f0:], qk_tiles[i // 4][64:128, isl],
                    qk2_tiles[j][64:128, f0:], start=True, stop=True,
                    tile_position=(64, 0),
                )
        e = expp.tile([128, 2, TJ], BF16, tag="e", name=f"e{j}_{b}")
        # batched exp across both PSUM banks; the steep staircase pair
        # (f0 >= 256 in both slots) only needs the [256:] columns.
        t0 = 256 if min(f0s) >= 256 else 0
        nc.scalar.activation(e[:, :, t0:], grp[:, :, t0:], EXP, scale=0.125)
        # zero the strict-upper triangle of each diagonal 128-wide window
        for slot in range(2):
            r = 2 * b + slot - 4 * j
            if r >= 0:
                f0 = 128 * r if r > 0 else 0
                nc.vector.tensor_mul(
                    e[:, slot, f0:f0 + 128], e[:, slot, f0:f0 + 128], mask01
                )
        etiles[(j, b)] = e

    def emit_pv_mms(j, c):
        m = 4 * j + c
        po = ps_o.tile([128, H + 1], F32, tag="po", name=f"po{j}_{c}")
        for i in range(m + 1):
            nc.tensor.matmul(
                po[:], etiles[(j, i // 2)][:, i % 2, c * 128:(c + 1) * 128],
                v_tiles[i // 4][:, i % 4, :],
                start=(i == 0), stop=(i == m),
            )
        return po

    def emit_pv_finish(j, c, po, engine=None):
        rec = small.tile([128, 1], F32, tag="rec", name=f"rec{j}_{c}")
        nc.vector.reciprocal(rec[:], po[:, H:H + 1])
        osb = outp.tile([128, H], BF16, tag="o", name=f"osb{j}_{c}")
        nc.vector.tensor_scalar_mul(osb[:], po[:, 0:H], rec[:])
        (engine or nc.sync).dma_start(out[j][c], osb[:])

    def emit_pv_window(j, c):
        """PV + normalize + store for output window c of column j."""
        emit_pv_finish(j, c, emit_pv_mms(j, c))

    # Emission order == static-scheduler priority: qk projections + score
    # pairs first (they feed ScalarE, the bottleneck of the steady
    # phase); each column's v projection is demoted to just after its
    # score pairs; PV windows after in chronological order -- the
    # scheduler slots PV matmuls into the exp-turnaround stalls on its
    # own.
    # v_j is demoted below column j+1's score pairs: on the in-order PE
    # stream, v matmuls emitted earlier would delay the score pairs that
    # feed ScalarE (the steady-phase bottleneck).  In the exp-paced
    # endgame (column 3) the PE runs short of real work between score
    # pairs, the HAM gate drops to 4/8, and the PV tail runs at half
    # clock -- tail filler matmuls (fresh ps_s tiles, so pool rotation
    # stays in emission order) keep the activity monitor fed.
    def emit_tail_filler(b):
        tw = ps_s.tile([128, 2, TJ], F32, tag="s", name=f"tw{b}")
        nc.tensor.matmul(tw[:, 0, :], warm[:, 0:128], warm[:],
                         start=True, stop=True)

    for j in range(NJ):
        emit_qk(j)
        for b in range(2 * j + 2):
            emit_score_pair(j, b)
            if j == NJ - 1 and b >= 3:
                emit_tail_filler(b)
        if j >= 1:
            emit_v(j - 1)
    emit_v(NJ - 1)
    for j in range(NJ - 1):
        for c in range(4):
            emit_pv_window(j, c)
    emit_pv_window(NJ - 1, 0)
    emit_pv_window(NJ - 1, 1)
    po2 = emit_pv_mms(NJ - 1, 2)
    po3 = emit_pv_mms(NJ - 1, 3)
    emit_pv_finish(NJ - 1, 2, po2)
    # last store via ScalarE HWDGE (idle by now) so the two final
    # completion receipts overlap instead of queueing
    emit_pv_finish(NJ - 1, 3, po3, engine=nc.scalar)

    ctx.close()


_NC_CACHE = None


def build_nc():
    global _NC_CACHE
    if _NC_CACHE is not None:
        return _NC_CACHE
    nc = bacc.Bacc(
        "TRN2", target_bir_lowering=False, debug=False,
        enable_asserts=False, num_devices=NCORES,
    )
    xp_dram = nc.dram_tensor("xp", [NJ, 128, 8, TJ], BF16, kind="ExternalInput").ap()
    wp_dram = nc.dram_tensor("wp", [128, WCW], BF16, kind="ExternalInput").ap()
    out = nc.dram_tensor("out", [NJ, 4, 128, H], BF16, kind="ExternalOutput").ap()
    with tile.TileContext(nc) as tc:
        build_kernel(tc, out, xp_dram, wp_dram)
    nc.finalize()
    _NC_CACHE = nc
    return nc


def _const_cst():
    p = np.arange(128)
    idn2 = np.tile(np.eye(64, dtype=np.float32), (2, 1))        # [128, 64]
    mask01 = (p[:, None] <= p[None, :]).astype(np.float32)      # [128, 128]
    return np.ascontiguousarray(
        np.concatenate([idn2, mask01], axis=1)
    ).astype(ml_dtypes.bfloat16)


def _marshal(x_b: np.ndarray):
    # xp[j, p, ec, t'] = x[j*TJ + t', ec*128 + p], cast bf16
    return np.ascontiguousarray(
        x_b.reshape(NJ, TJ, 8, 128).transpose(0, 3, 2, 1)
    ).astype(ml_dtypes.bfloat16)


def _install_profile_hook():
    """The agent image lacks ``antenv.axon_hooks``; inject a shim so
    run_bass_kernel_spmd(trace=True) can reach the axon NTFF profiler."""
    import types

    if "antenv.axon_hooks" not in sys.modules:
        mod = types.ModuleType("antenv.axon_hooks")
        holder = {}
        mod.set_axon_ntff_profile_hook = lambda h: holder.__setitem__("h", h)
        mod.get_axon_ntff_profile_hook = lambda: holder.get("h")
        sys.modules["antenv.axon_hooks"] = mod
    from trn_agent_boot.trn_boot import _ntff_profile_via_ctypes

    hook = _ntff_profile_via_ctypes("/opt/axon/libaxon_pjrt.so")
    sys.modules["antenv.axon_hooks"].set_axon_ntff_profile_hook(hook)
    # no fish bucket in this container -- keep artifacts local
    from concourse import bass_utils as bu

    bu.upload_artifacts = lambda tmpdir: tmpdir


def run(inputs: dict, trace: bool = False, tmpdir: str | None = None):
    """Returns (out [8, 2048, 64] f32, exec_time_ns or None)."""
    x = np.asarray(inputs["x"], dtype=np.float32)
    # wp[p, ec, r, h] = W_r[ec*128 + p, h], cast bf16
    wqkv = np.stack([np.asarray(inputs["Wq"]), np.asarray(inputs["Wk"]),
                     np.asarray(inputs["Wv"])]).astype(np.float32)
    w_pre = np.ascontiguousarray(
        wqkv.reshape(3, 8, 128, H).transpose(2, 1, 0, 3)
    ).astype(ml_dtypes.bfloat16)
    nc = build_nc()
    if trace:
        _install_profile_hook()
    wc = np.concatenate(
        [w_pre.reshape(128, 1536), _const_cst()], axis=1
    )
    in_maps = [{"xp": _marshal(x[b]), "wp": wc} for b in range(B)]
    res = run_bass_kernel_spmd(
        nc, in_maps, core_ids=list(range(NCORES)), trace=trace, tmpdir=tmpdir
    )
    # out[j, c, p, h]: (j, c, p) lexicographic == t = j*512 + c*128 + p
    out = np.stack([
        res.results[b]["out"].reshape(T, H) for b in range(B)
    ]).astype(np.float32)
    return out, res.exec_time_ns


def kernel(**inputs) -> np.ndarray:
    out, _ = run(inputs)
    return out


if __name__ == "__main__":
    rng = np.random.default_rng(0)
    ins = {
        "x": rng.standard_normal((B, T, E), dtype=np.float32),
        "Wq": rng.uniform(-1 / 32, 1 / 32, (E, H)).astype(np.float32),
        "Wk": rng.uniform(-1 / 32, 1 / 32, (E, H)).astype(np.float32),
        "Wv": rng.uniform(-1 / 32, 1 / 32, (E, H)).astype(np.float32),
    }
    o, ns = run(ins, trace=False)
    print("out", o.shape, o.dtype, "exec_ns", ns)
